# revision 1
# baseline (speedup 1.0000x reference)
"""Multi-head self-attention TRN2 kernel (16 heads, D=1024, x:[2,2048,1024]).

Sharding: 8 cores = 2 (batch) x 4 (head groups of 4 heads).
Each core computes, for its batch b and heads hg*4..hg*4+3:
    qT/kT = (x_b @ wq/wk + b)^T in head-dim-major layout  [256, 2048]
    v     = x_b @ wv + bv (token-major, ones-augmented)   [2048, 4, 65]
    per head, per q-chunk: scoresT = kT_h^T-free matmuls  [k=2048, q=512]
    exp via ACT (scale=1/8, no max subtraction: |s|/8 < 10 for randn inputs)
    oT/sums via ones-augmented AV matmul, softmax-normalize via
    DVE reciprocal_approx_accurate + gpsimd partition_broadcast
    partial_out = oT^T @ wo_rows + bo  (bo only on core with hg==0)
Host sums the 4 partials per batch (the tensor-parallel all-reduce).

All matmuls run as float32r (full-rate fp32, ~1.5e-4/dot rounding).
"""

import os
import sys
from contextlib import ExitStack

import numpy as np

for _p in ("/opt/trn_rl_repo", os.path.expanduser("~/.axon_site/_ro/trn_rl_repo")):
    if os.path.isdir(_p) and _p not in sys.path:
        sys.path.insert(0, _p)

import concourse.bass as bass  # noqa: E402
import concourse.mybir as mybir  # noqa: E402
import concourse.tile as tile  # noqa: E402
from concourse import bacc, library_config  # noqa: E402
from concourse.bass_utils import run_bass_kernel_spmd  # noqa: E402

f32 = mybir.dt.float32
f32r = mybir.dt.float32r
P = 128


def build_core_program(D=1024, TOK=2048, NH=4, num_devices=8):
    """One core's program: heads-of-one-batch slice of the attention layer.

    D: hidden size; TOK: sequence length; NH: heads per core (head dim 64).
    """
    DH = 64
    KD = D // P          # hidden-dim 128-chunks
    NQ = TOK // 512      # 512-wide q chunks
    NT = TOK // P        # 128-wide token chunks
    DC = NH * DH         # per-core head dims (q/k/v width)
    MQ = max(DC // P, 1)  # 128-row chunks of qT/kT/oT
    HPC = P // DH        # heads per 128-row chunk (2)
    OW = min(512, D)     # output column chunk width
    NO = D // OW         # output column chunks

    nc = bacc.Bacc("TRN2", target_bir_lowering=False, debug=False,
                   num_devices=num_devices)

    xT_d = nc.declare_dram_parameter("xT", [D, TOK], f32r, isOutput=False)
    wq_d = nc.declare_dram_parameter("wq", [D, DC], f32r, isOutput=False)
    wk_d = nc.declare_dram_parameter("wk", [D, DC], f32r, isOutput=False)
    wv_d = nc.declare_dram_parameter("wv", [D, DC], f32r, isOutput=False)
    wo_d = nc.declare_dram_parameter("wo", [DC, D], f32r, isOutput=False)
    bq_d = nc.declare_dram_parameter("bq", [P, MQ], f32, isOutput=False)
    bk_d = nc.declare_dram_parameter("bk", [P, MQ], f32, isOutput=False)
    bv_d = nc.declare_dram_parameter("bv", [P, DC], f32, isOutput=False)
    bo_d = nc.declare_dram_parameter("bo", [P, D], f32, isOutput=False)
    onesr_d = nc.declare_dram_parameter("onesr", [P, NH], f32r, isOutput=False)
    out_d = nc.declare_dram_parameter("out", [TOK, D], f32, isOutput=True)

    with tile.TileContext(nc) as tc, ExitStack() as ctx:
        persist = ctx.enter_context(tc.tile_pool(name="persist", bufs=1))
        phasexq = ctx.enter_context(tc.tile_pool(name="phasexq", bufs=1))
        phaseb_cm = tc.tile_pool(name="phaseb", bufs=1)
        phaseb = phaseb_cm.__enter__()
        psc = ctx.enter_context(tc.tile_pool(name="psc", bufs=2, space="PSUM"))
        pacc = ctx.enter_context(tc.tile_pool(name="pacc", bufs=2, space="PSUM"))
        nc.gpsimd.load_library(library_config.attn)

        # ---- phase A: load everything (weights first; xT in the
        # order the kT projection consumes it) -------------------------
        xT_sb = phasexq.tile([P, KD, TOK], f32r)
        wq_sb = phasexq.tile([P, KD, DC], f32r)
        wk_sb = phaseb.tile([P, KD, DC], f32r)
        wv_sb = phaseb.tile([P, KD, DC], f32r)
        nc.sync.dma_start(wk_sb[:], wk_d.rearrange("(ko ki) n -> ki ko n", ki=P))
        nc.gpsimd.dma_start(wq_sb[:], wq_d.rearrange("(ko ki) n -> ki ko n", ki=P))
        nc.gpsimd.dma_start(wv_sb[:], wv_d.rearrange("(ko ki) n -> ki ko n", ki=P))
        wo_sb = persist.tile([P, MQ, D], f32r)
        nc.gpsimd.dma_start(wo_sb[:], wo_d.rearrange("(mo mi) n -> mi mo n", mi=P))

        bq_sb = persist.tile([P, MQ], f32)
        bk_sb = persist.tile([P, MQ], f32)
        bv_sb = phaseb.tile([P, DC], f32)
        bo_sb = persist.tile([P, D], f32)
        nc.gpsimd.dma_start(bq_sb[:], bq_d[:])
        nc.gpsimd.dma_start(bk_sb[:], bk_d[:])
        nc.gpsimd.dma_start(bv_sb[:], bv_d[:])
        nc.gpsimd.dma_start(bo_sb[:], bo_d[:])
        onesr_sb = persist.tile([P, NH], f32r)
        nc.gpsimd.dma_start(onesr_sb[:], onesr_d[:])
        for n in range(NQ):
            for ko in range(KD):
                nc.sync.dma_start(
                    xT_sb[:, ko, n * 512:(n + 1) * 512],
                    xT_d[ko * P:(ko + 1) * P, n * 512:(n + 1) * 512])

        # ---- phase B: kT and v projections (whole-sequence deps) -----
        qT_sb = persist.tile([P, MQ, TOK], f32r)
        kT_sb = persist.tile([P, MQ, TOK], f32r)

        def proj_block(w_sb, b_sb, t_sb, m, n, tag="acc"):
            ps = pacc.tile([P, 512], f32, tag=tag, name="ps")
            for ko in range(KD):
                nc.tensor.matmul(
                    ps[:], w_sb[:, ko, m * P:(m + 1) * P],
                    xT_sb[:, ko, n * 512:(n + 1) * 512],
                    start=(ko == 0), stop=(ko == KD - 1))
            nc.vector.tensor_tensor(
                t_sb[:, m, n * 512:(n + 1) * 512], ps[:],
                b_sb[:, m:m + 1].to_broadcast([P, 512]),
                mybir.AluOpType.add)

        for m in range(MQ):
            for n in range(NQ):
                proj_block(wk_sb, bk_sb, kT_sb, m, n)
        for m in range(MQ):
            proj_block(wq_sb, bq_sb, qT_sb, m, 0)

        # v token-major, per (token-chunk, head): [128, 65] with ones col
        v_sb = persist.tile([P, NT, NH, DH + 1], f32r)
        for t in range(NT):
            nc.vector.tensor_copy(v_sb[:, t, :, DH:DH + 1],
                                  onesr_sb[:, :, None])
            ps = pacc.tile([P, DC], f32, tag="acc")
            for ko in range(KD):
                nc.tensor.matmul(
                    ps[:], xT_sb[:, ko, t * P:(t + 1) * P], wv_sb[:, ko, :],
                    start=(ko == 0), stop=(ko == KD - 1))
            nc.vector.tensor_tensor(
                v_sb[:, t, :, 0:DH],
                ps.rearrange("p (h d) -> p h d", h=NH),
                bv_sb.rearrange("p (h d) -> p h d", h=NH),
                mybir.AluOpType.add)

        # ---- phase C: attention + per-block output projection --------
        # Heads are processed in pairs occupying PE row strips 0-63 /
        # 64-127 so adjacent score matmuls (K=64) pack into the array.
        # AV matmuls for group g are emitted after scores of group g+1
        # so the in-order PE queue keeps running while ACT does exp(g).
        phaseb_cm.__exit__(None, None, None)
        work = ctx.enter_context(tc.tile_pool(name="work", bufs=3))
        oT_sb = persist.tile([P, MQ, TOK], f32r)
        G = NT // 2

        def emit_scores(pair, n, g, scs):
            qs = slice(n * 512, (n + 1) * 512)
            for j in range(2):
                kk = g * 2 + j
                for h in pair:
                    hm = h // HPC
                    hr = (h % HPC) * DH
                    nc.tensor.matmul(
                        scs[h][:, j, :],
                        kT_sb[hr:hr + DH, hm, kk * P:(kk + 1) * P],
                        qT_sb[hr:hr + DH, hm, qs],
                        start=True, stop=True)

        def emit_av(pair, g, avs, exs):
            for h in pair:
                for j in range(2):
                    nc.tensor.matmul(
                        avs[h], v_sb[:, g * 2 + j, h, :], exs[h][:, j, :],
                        start=(g == 0 and j == 0),
                        stop=(g == G - 1 and j == 1))

        def emit_oproj(n):
            for t in range(4):
                tok = n * 4 + t
                for nn in range(NO):
                    ns = slice(nn * OW, (nn + 1) * OW)
                    op = pacc.tile([P, OW], f32, tag="opj", name="op")
                    for m in range(MQ):
                        nc.tensor.matmul(
                            op[:], oT_sb[:, m, tok * P:(tok + 1) * P],
                            wo_sb[:, m, ns],
                            start=(m == 0), stop=(m == MQ - 1))
                    ou = work.tile([P, OW], f32, tag="out", name="ou")
                    nc.vector.tensor_tensor(
                        ou[:], op[:], bo_sb[:, ns], mybir.AluOpType.add)
                    nc.sync.dma_start(out_d[tok * P:(tok + 1) * P, ns], ou[:])

        for n in range(NQ):
            qs = slice(n * 512, (n + 1) * 512)
            for hp in range(NH // HPC):
                if hp == 1 and n > 0:
                    emit_oproj(n - 1)
                pair = [hp * HPC + i for i in range(HPC)]
                avs = {h: pacc.tile([DH + 1, 512], f32, tag="acc",
                                    name=f"av{h}") for h in pair}
                prev = None
                for g in range(G):
                    scs = {h: psc.tile([P, 2, 512], f32, tag="sc",
                                       name=f"sc{h}") for h in pair}
                    emit_scores(pair, n, g, scs)
                    exs = {}
                    for h in pair:
                        ex = work.tile([P, 2, 512], f32r, tag=f"ex{h % HPC}", name="ex")
                        nc.scalar.activation(
                            ex[:], scs[h][:],
                            mybir.ActivationFunctionType.Exp, scale=0.125)
                        exs[h] = ex
                    if prev is not None:
                        emit_av(pair, g - 1, avs, prev)
                    prev = exs
                emit_av(pair, G - 1, avs, prev)
                # drain + softmax-normalize per head of the pair
                for h in pair:
                    hm = h // HPC
                    hr = (h % HPC) * DH
                    od = oT_sb[hr:hr + DH, hm, qs]
                    nc.vector.tensor_copy(od, avs[h][0:DH, :])
                    srow = work.tile([1, 512], f32, tag="srow")
                    nc.vector.tensor_copy(srow[:], avs[h][DH:DH + 1, :])
                    r32 = work.tile([1, 512], f32, tag="r32")
                    scr = work.tile([1, 512], f32, tag="scr")
                    nc.vector.reciprocal_approx_accurate(r32[:], srow[:],
                                                         scr[:])
                    bc = work.tile([P, 512], f32, tag="bc")
                    nc.gpsimd.partition_broadcast(bc[:], r32[:])
                    nc.vector.tensor_tensor(od, od, bc[hr:hr + DH, :],
                                            mybir.AluOpType.mult)
                if n + 1 < NQ:
                    proj_block(wq_sb, bq_sb, qT_sb, hp, n + 1, tag="opj")
        emit_oproj(NQ - 1)
    return nc


_CACHE = {}
LAST_RESULTS = None


def _get_compiled():
    if "nc" not in _CACHE:
        nc = build_core_program()
        nc.compile()
        _CACHE["nc"] = nc
    return _CACHE["nc"]


def kernel(x, wq, bq, wk, bk, wv, bv, wo, bo):
    global LAST_RESULTS
    x = np.asarray(x, np.float32)
    wq, bq = np.asarray(wq, np.float32), np.asarray(bq, np.float32)
    wk, bk = np.asarray(wk, np.float32), np.asarray(bk, np.float32)
    wv, bv = np.asarray(wv, np.float32), np.asarray(bv, np.float32)
    wo, bo = np.asarray(wo, np.float32), np.asarray(bo, np.float32)
    B, TOK, D = x.shape          # (2, 2048, 1024)
    NH, DH = 4, 64               # heads per core, head dim
    DC = NH * DH                 # 256
    MQ = DC // P                 # 2

    nc = _get_compiled()

    bo_rep = np.ascontiguousarray(np.tile(bo[None, :], (P, 1)))
    zeros_bo = np.zeros_like(bo_rep)
    ones_r = np.ones((P, NH), np.float32)

    in_maps = []
    for c in range(8):
        b, hg = c // 4, c % 4
        sl = slice(hg * DC, (hg + 1) * DC)
        in_maps.append({
            "xT": np.ascontiguousarray(x[b].T),
            "wq": np.ascontiguousarray(wq[:, sl]),
            "wk": np.ascontiguousarray(wk[:, sl]),
            "wv": np.ascontiguousarray(wv[:, sl]),
            "wo": np.ascontiguousarray(wo[sl, :]),
            "bq": np.ascontiguousarray(bq[sl].reshape(MQ, P).T),
            "bk": np.ascontiguousarray(bk[sl].reshape(MQ, P).T),
            "bv": np.ascontiguousarray(np.tile(bv[None, sl], (P, 1))),
            "bo": bo_rep if hg == 0 else zeros_bo,
            "onesr": ones_r,
        })

    trace = os.environ.get("KERNEL_TRACE", "0") == "1"
    res = run_bass_kernel_spmd(nc, in_maps, core_ids=list(range(8)),
                               trace=trace)
    LAST_RESULTS = res
    outs = [res.results[c]["out"] for c in range(8)]
    y = np.stack([sum(outs[0:4]), sum(outs[4:8])], axis=0)
    return np.ascontiguousarray(y, dtype=np.float32)



# revision 34
# speedup vs baseline: 1.2101x; 1.2101x over previous
"""Multi-head self-attention TRN2 kernel (16 heads, D=1024, x:[2,2048,1024]).

Sharding: 8 cores = 2 (batch) x 4 (head groups of 4 heads). Host sums the
4 partials per batch (tensor-parallel all-reduce) and adds bo.

Per-core pipeline (all matmul operands bf16, accumulation f32 in PSUM):
  kT/qT = (x @ wq/wk)^T + b     head-dim-major [256, 2048] bf16
  v     = x @ wv + bv           token-major [2048, 4, 65] bf16, ones column
  scores = kT_h^T-strips @ qT   [128 keys, 512 q] f32 PSUM per chunk
  ex    = exp(s/8): ACT engine (bf16 out) for 3 of 4 heads; DVE Schraudolph
          (affine + uint16 trunc = bf16 exp bits) for the 4th head
  o^T   = AV in q-major orientation: out [128 q, 65] = ex_chunk^T @ v_chunk,
          4 q-subblocks packed per PSUM bank (single start=True zeroes bank)
  norm  = DVE divide by the ones-column sums (per-partition scalar)
  oT    = XBAR DMA transpose [128 q, 128 dh-pair] -> [128, 128] into oT_sb
  out   = oT^T @ wo per 128-token chunk, DMA'd to DRAM straight from PSUM
"""

import os
import sys
from contextlib import ExitStack

import numpy as np

for _p in ("/opt/trn_rl_repo", os.path.expanduser("~/.axon_site/_ro/trn_rl_repo")):
    if os.path.isdir(_p) and _p not in sys.path:
        sys.path.insert(0, _p)

import ml_dtypes  # noqa: E402

import concourse.bass as bass  # noqa: E402
import concourse.mybir as mybir  # noqa: E402
import concourse.tile as tile  # noqa: E402
from concourse import bacc  # noqa: E402
from concourse.bass_utils import run_bass_kernel_spmd  # noqa: E402

f32 = mybir.dt.float32
bf16 = mybir.dt.bfloat16
u16 = mybir.dt.uint16
P = 128

# Schraudolph exp-as-bf16-bits: bits = A*s_raw + B, trunc to uint16.
# A = 128*log2(e)*0.125 (the 1/8 softmax scale folded in); B centers the
# piecewise-linear ripple (minimax) and compensates truncation.
SCH_A = 128.0 * float(np.log2(np.e)) * 0.125
SCH_B = 128.0 * 127.0 - 6.86


def build_core_program(D=1024, TOK=2048, NH=4, num_devices=8):
    """One core's program: 4 heads of one batch of the attention layer."""
    DH = 64
    KD = D // P          # hidden-dim 128-chunks (8)
    NQ = TOK // 512      # 512-wide q blocks (4)
    NT = TOK // P        # 128-wide token chunks (16)
    DC = NH * DH         # per-core head dims (256)
    MQ = DC // P         # 128-row chunks of qT/kT/oT (2)
    HPC = P // DH        # heads per 128-row chunk (2)
    OW = 512             # output column chunk width
    NO = D // OW         # output column chunks (2)
    G = NT // 2          # key-pair groups per stage (8)

    nc = bacc.Bacc("TRN2", target_bir_lowering=False, debug=False,
                   num_devices=num_devices)

    KC = KD // 2         # 256-row DoubleRow chunks (4)
    fp8 = mybir.dt.float8e4
    x8_d = nc.declare_dram_parameter("x8", [P, KC, 2, TOK], fp8, isOutput=False)
    xr_d = nc.declare_dram_parameter("xr", [P, KC, 2, TOK], fp8, isOutput=False)
    xs_d = nc.declare_dram_parameter("xs", [P, KC, 2, TOK], fp8, isOutput=False)
    w8_d = {}
    wr_d = {}
    for nm in ("q", "k", "v"):
        w8_d[nm] = nc.declare_dram_parameter(
            f"w8{nm}", [P, KC, 2, DC], fp8, isOutput=False)
        wr_d[nm] = nc.declare_dram_parameter(
            f"wr{nm}", [P, KC, 2, DC], fp8, isOutput=False)
    wo_d = nc.declare_dram_parameter("wo", [P, MQ, D], bf16, isOutput=False)
    bq_d = nc.declare_dram_parameter("bq", [P, MQ], f32, isOutput=False)
    bk_d = nc.declare_dram_parameter("bk", [P, MQ], f32, isOutput=False)
    bv_d = nc.declare_dram_parameter("bv", [P, DC], f32, isOutput=False)
    out_d = nc.declare_dram_parameter("out", [TOK, D], bf16, isOutput=True)

    with tile.TileContext(nc) as tc, ExitStack() as ctx:
        persist = ctx.enter_context(tc.tile_pool(name="persist", bufs=1))
        work = ctx.enter_context(tc.tile_pool(name="work", bufs=3))
        psp = ctx.enter_context(tc.tile_pool(name="psp", bufs=2, space="PSUM"))

        # ---- phase A: loads ------------------------------------------
        w8_sb = {nm: persist.tile([P, KC, 2, DC], fp8, name=f"w8{nm}")
                 for nm in ("q", "k", "v")}
        wr_sb = {nm: persist.tile([P, KC, 2, DC], fp8, name=f"wr{nm}")
                 for nm in ("q", "k", "v")}
        wo_sb = persist.tile([P, MQ, D], bf16)
        bq_sb = persist.tile([P, MQ], f32)
        bk_sb = persist.tile([P, MQ], f32)
        bv_sb = persist.tile([P, DC], f32)
        x8_sb = persist.tile([P, KC, 2, TOK], fp8)
        xr_sb = persist.tile([P, KC, 2, TOK], fp8)
        xs_sb = persist.tile([P, KC, 2, TOK], fp8)

        # startup-critical transfers first (kT0/qT0 2-set inputs), then the
        # stream in stage-(0,*) consumption order
        nc.gpsimd.dma_start(w8_sb["k"][:], w8_d["k"][:])
        nc.sync.dma_start(x8_sb[:, :, :, 0:512], x8_d[:, :, :, 0:512])
        nc.gpsimd.dma_start(w8_sb["q"][:], w8_d["q"][:])
        nc.sync.dma_start(xr_sb[:, :, :, 0:512], xr_d[:, :, :, 0:512])
        nc.gpsimd.dma_start(bk_sb[:], bk_d[:])
        nc.gpsimd.dma_start(bq_sb[:], bq_d[:])
        nc.gpsimd.dma_start(w8_sb["v"][:], w8_d["v"][:])
        nc.gpsimd.dma_start(wr_sb["v"][:], wr_d["v"][:])
        nc.gpsimd.dma_start(bv_sb[:], bv_d[:])
        nc.sync.dma_start(xs_sb[:, :, :, 0:512], xs_d[:, :, :, 0:512])
        nc.gpsimd.dma_start(wr_sb["k"][:], wr_d["k"][:])
        nc.gpsimd.dma_start(wr_sb["q"][:], wr_d["q"][:])
        for n in range(1, NQ):
            ns = slice(n * 512, (n + 1) * 512)
            for t_sb, t_d in ((x8_sb, x8_d), (xr_sb, xr_d), (xs_sb, xs_d)):
                nc.sync.dma_start(t_sb[:, :, :, ns], t_d[:, :, :, ns])
        nc.gpsimd.dma_start(wo_sb[:], wo_d[:])

        v_sb = persist.tile([P, NT, NH, DH + 1], bf16)
        nc.vector.memset(v_sb[:, :, :, DH:DH + 1], 1.0)

        qT_sb = persist.tile([P, MQ, TOK], bf16)
        kT_sb = persist.tile([P, MQ, TOK], bf16)
        oT_sb = persist.tile([P, MQ, TOK], bf16)

        # ---- phase B: kT/qT0 for head-pair 0 only; the rest of the
        # projections are interleaved into the first two stages --------
        DR = mybir.MatmulPerfMode.DoubleRow
        PROJ_SETS = (("8", x8_sb), ("8", xr_sb), ("r", xs_sb))

        def proj_block(nm, b_sb, t_sb, m, n, nsets=3):
            # t = (x8@w8 + xr8@w8 + x8s@wr8s) + b, fp8e4 DoubleRow
            ps = psp.tile([P, 512], f32, tag="acc", name="ps")
            ns = slice(n * 512, (n + 1) * 512)
            wsb = {"8": w8_sb[nm], "r": wr_sb[nm]}
            for si, (wv_, xv) in enumerate(PROJ_SETS[:nsets]):
                for c in range(KC):
                    nc.tensor.matmul(
                        ps[:], wsb[wv_][:, c, :, m * P:(m + 1) * P],
                        xv[:, c, :, ns], perf_mode=DR,
                        start=(si == 0 and c == 0),
                        stop=(si == nsets - 1 and c == KC - 1))
            nc.vector.tensor_tensor(
                t_sb[:, m, ns], ps[:],
                b_sb[:, m:m + 1].to_broadcast([P, 512]),
                mybir.AluOpType.add)

        # first two blocks skip the w-residual set: 3 fewer DMA transfers
        # on the startup critical path, ~0.03% extra noise on those columns
        proj_block("k", bk_sb, kT_sb, 0, 0, nsets=2)
        proj_block("q", bq_sb, qT_sb, 0, 0, nsets=2)

        def emit_v_chunk(t, vp):
            ps = psp.tile([P, P], f32, tag="acc", name="vps")
            tsl = slice(t * P, (t + 1) * P)
            vsl = slice(vp * P, (vp + 1) * P)
            wsb = {"8": w8_sb["v"], "r": wr_sb["v"]}
            for si, (wv_, xv) in enumerate(PROJ_SETS):
                for c in range(KC):
                    nc.tensor.matmul(
                        ps[:], xv[:, c, :, tsl], wsb[wv_][:, c, :, vsl],
                        perf_mode=DR,
                        start=(si == 0 and c == 0),
                        stop=(si == 2 and c == KC - 1))
            nc.vector.tensor_tensor(
                v_sb[:, t, 2 * vp:2 * vp + 2, 0:DH],
                ps.rearrange("p (h d) -> p h d", h=HPC),
                bv_sb[:, vsl].rearrange("p (h d) -> p h d", h=HPC),
                mybir.AluOpType.add)

        # ---- phase C: attention + output projection ------------------
        def emit_oproj_tok(tok):
            ts = slice(tok * P, (tok + 1) * P)
            for nn in range(NO):
                ns = slice(nn * OW, (nn + 1) * OW)
                op = psp.tile([P, OW], f32, tag="acc", name="op")
                for m in range(MQ):
                    nc.tensor.matmul(
                        op[:], oT_sb[:, m, ts], wo_sb[:, m, ns],
                        start=(m == 0), stop=(m == MQ - 1))
                ou = work.tile([P, OW], bf16, tag="ou", name="ou")
                nc.vector.tensor_copy(ou[:], op[:])
                nc.gpsimd.dma_start(out_d[ts, ns], ou[:])

        def emit_oproj(n):
            for t in range(4):
                emit_oproj_tok(n * 4 + t)

        for n in range(NQ):
            qs = slice(n * 512, (n + 1) * 512)
            for hp in range(MQ):
                pair = [hp * HPC, hp * HPC + 1]
                avs = {}
                for h in pair:
                    avs[h] = psp.tile([P, 4, DH + 1], f32, tag="av",
                                      padded_shape=[P, 4, P], name=f"av{h}")

                def emit_av(g, exs, which):
                    for h in which:
                        ex = exs[h]
                        for j in range(2):
                            for q4 in range(4):
                                nc.tensor.matmul(
                                    avs[h][:, q4, :],
                                    ex[:, j, q4 * P:(q4 + 1) * P],
                                    v_sb[:, 2 * g + j, h, :],
                                    start=(g == 0 and j == 0 and q4 == 0),
                                    stop=(g == G - 1 and j == 1),
                                    skip_group_check=True)

                if n == 0 and hp == 1:
                    proj_block("k", bk_sb, kT_sb, 1, 0)
                exs_hist = {}
                for g in range(G):
                    if n == 0 and g % 2 == 0 and g > 0:
                        # stream the kT block feeding this key-group pair
                        proj_block("k", bk_sb, kT_sb, hp, g // 2)
                    cur_exs = {}
                    for i, h in enumerate(pair):
                        sc = psp.tile([P, 2, 512], f32, tag="sc",
                                      name=f"sc{h}")
                        hm, hr = h // HPC, (h % HPC) * DH
                        for j in range(2):
                            kk = g * 2 + j
                            nc.tensor.matmul(
                                sc[:, j, :],
                                kT_sb[hr:hr + DH, hm, kk * P:(kk + 1) * P],
                                qT_sb[hr:hr + DH, hm, qs],
                                start=True, stop=True)
                        # exp: DVE Schraudolph for ~5/16 of (head, key-group)
                        # slices; ACT exact exp otherwise
                        if (i == 1 and g % 2 == 0) or (i == 0 and g == 3):
                            ex = work.tile([P, 2, 512], u16, bufs=4,
                                           tag=f"ex{i}", name="exu")
                            nc.vector.tensor_scalar(
                                ex[:], sc[:], SCH_A, SCH_B,
                                mybir.AluOpType.mult, mybir.AluOpType.add)
                            cur_exs[h] = ex.bitcast(bf16)
                        else:
                            ex = work.tile([P, 2, 512], bf16, bufs=4,
                                           tag=f"ex{i}", name="exb")
                            nc.scalar.activation(
                                ex[:], sc[:],
                                mybir.ActivationFunctionType.Exp, scale=0.125)
                            cur_exs[h] = ex
                        # AV lagged two groups behind scores/exp to absorb
                        # exp latency; halved between the heads' emissions
                        if g >= 2:
                            emit_av(g - 2, exs_hist[g - 2], [pair[i]])
                    if n == 0:
                        # v projections streamed just ahead of their AV use
                        emit_v_chunk(2 * g, hp)
                        emit_v_chunk(2 * g + 1, hp)
                        if hp == 0 and g == G - 1:
                            proj_block("q", bq_sb, qT_sb, 1, 0)
                    elif hp == 1 and g < 4:
                        # previous block's output projection, one token
                        # chunk per key-group to spread PSUM slot reuse
                        emit_oproj_tok((n - 1) * 4 + g)
                    exs_hist[g] = cur_exs
                emit_av(G - 2, exs_hist[G - 2], pair)
                emit_av(G - 1, exs_hist[G - 1], pair)
                # normalize + transpose into oT (+ last-block oproj tail)
                last = (n == NQ - 1 and hp == MQ - 1)
                onorm = work.tile([P, 4, P], bf16, tag="onorm", bufs=3,
                                  name="onorm")
                for i, h in enumerate(pair):
                    rcp = work.tile([P, 4, 1], f32, tag="rcp", bufs=4,
                                    name="rcp")
                    nc.vector.reciprocal_approx_fast(
                        rcp[:], avs[h][:, :, DH:DH + 1])
                    nc.vector.tensor_tensor(
                        onorm[:, :, i * DH:(i + 1) * DH],
                        avs[h][:, :, 0:DH],
                        rcp.to_broadcast([P, 4, DH]),
                        mybir.AluOpType.mult)
                if n + 1 < NQ:
                    proj_block("q", bq_sb, qT_sb, hp, n + 1)
                for q4 in range(4):
                    nc.sync.dma_start_transpose(
                        oT_sb[:, hp, n * 512 + q4 * P: n * 512 + (q4 + 1) * P],
                        onorm[:, q4, :])
                    if last:
                        emit_oproj_tok(n * 4 + q4)
    return nc


_CACHE = {}
LAST_RESULTS = None


def _get_compiled():
    if "nc" not in _CACHE:
        nc = build_core_program()
        nc.compile()
        _CACHE["nc"] = nc
    return _CACHE["nc"]


def kernel(x, wq, bq, wk, bk, wv, bv, wo, bo):
    global LAST_RESULTS
    x = np.asarray(x, np.float32)
    wq, bq = np.asarray(wq, np.float32), np.asarray(bq, np.float32)
    wk, bk = np.asarray(wk, np.float32), np.asarray(bk, np.float32)
    wv, bv = np.asarray(wv, np.float32), np.asarray(bv, np.float32)
    wo, bo = np.asarray(wo, np.float32), np.asarray(bo, np.float32)
    B, TOK, D = x.shape          # (2, 2048, 1024)
    NH, DH = 4, 64               # heads per core, head dim
    DC = NH * DH                 # 256
    MQ = DC // P                 # 2
    KD = D // P                  # 8
    KC = KD // 2                 # 4
    BF = ml_dtypes.bfloat16
    E4 = ml_dtypes.float8_e4m3

    nc = _get_compiled()

    def chunk_rows(a, nchunk):
        # [R, C] -> [P, nchunk, C] with [p, c, :] = a[c*P + p, :]
        R, C = a.shape
        return np.ascontiguousarray(
            a.reshape(nchunk, P, C).transpose(1, 0, 2))

    def dr_chunks(a):
        # [R, C] -> [P, KC, 2, C] DoubleRow layout (row = c*256 + j*128 + p)
        c = chunk_rows(a, KD)  # [P, 8, C]
        return np.ascontiguousarray(
            c.reshape(P, KC, 2, a.shape[1]))

    def fp8_sets(a):
        # returns (a8, ar8, a8s): value + residual + 2^-5-scaled copies
        a8 = a.astype(E4)
        ar = (a - a8.astype(np.float32)).astype(E4)
        as_ = (a * 2.0 ** -5).astype(E4)
        return a8, ar, as_

    in_maps = []
    x8_b, xr_b, xs_b = [], [], []
    for b in range(B):
        x8, xr, xs = fp8_sets(x[b].T)  # [D, TOK]
        x8_b.append(dr_chunks(x8))
        xr_b.append(dr_chunks(xr))
        xs_b.append(dr_chunks(xs))
    for c in range(8):
        b, hg = c // 4, c % 4
        sl = slice(hg * DC, (hg + 1) * DC)
        m = {
            "x8": x8_b[b], "xr": xr_b[b], "xs": xs_b[b],
            "wo": chunk_rows(wo[sl, :], MQ).astype(BF),
            "bq": np.ascontiguousarray(bq[sl].reshape(MQ, P).T),
            "bk": np.ascontiguousarray(bk[sl].reshape(MQ, P).T),
            "bv": np.ascontiguousarray(np.tile(bv[None, sl], (P, 1))),
        }
        for nm, w in (("q", wq), ("k", wk), ("v", wv)):
            w8 = w[:, sl].astype(E4)
            wr = ((w[:, sl] - w8.astype(np.float32)) * 2.0 ** 5).astype(E4)
            m[f"w8{nm}"] = dr_chunks(w8)
            m[f"wr{nm}"] = dr_chunks(wr)
        in_maps.append(m)

    trace = os.environ.get("KERNEL_TRACE", "0") == "1"
    res = run_bass_kernel_spmd(nc, in_maps, core_ids=list(range(8)),
                               trace=trace)
    LAST_RESULTS = res
    outs = [res.results[c]["out"].astype(np.float32) for c in range(8)]
    y = np.stack([sum(outs[0:4]) + bo, sum(outs[4:8]) + bo], axis=0)
    return np.ascontiguousarray(y, dtype=np.float32)
